# revision 9
# baseline (speedup 1.0000x reference)
"""Causal GQA attention block (B=2, T=2048, C=2048, H=16, HKV=4, D=128, RoPE)
on 8 Trainium2 NeuronCores.

Sharding: core c handles batch b = c//4 and kv-group g = c%4 (4 q heads +
1 kv head per core).  The output projection is row-parallel: each core
produces a partial [T, C] contribution; the host sums the 4 partials per
batch.

v2 design (single fused jt-pipelined loop, fp16 storage):
  - all SBUF tensors fp16 (magnitudes here stay < 1e4, fp16 keeps per-elem
    rel err ~5e-4; matmuls run at 1 cycle/row like bf16).
  - x is host-transposed to xT [C, T]; q/k projections produce [D, T]
    head-transposed tiles; RoPE applied via host-side even/odd permutation
    folded into wq/wk + partition-swapped multiplies.
  - v is produced directly in [T, D] layout (x chunk as the stationary
    operand, wv as moving) -- no PE transposes at all.
  - scores are computed transposed (S.T tiles [s, t]); causal structure is
    exploited at fine grain: diagonal s-tiles only compute the t >= s part
    (moving dim shortened to 512-128r), with a single [128,128] triangular
    multiplicative mask for the crossing block.
  - softmax: no max-subtraction (scores are O(5), exp safe in fp32 PSUM);
    1/sqrt(D) folded into the exp's scale argument (free on ACT).
    The denominator is accumulated on DVE (fp16 adds of exp'd tiles) and
    partition-reduced+broadcast in one gpsimd partition_all_reduce -- no
    tensor-engine work.
  - the output projection for chunk jt-1 is interleaved into the attention
    h-loop of chunk jt (one tt row-block per head) so its matmuls fill the
    attention phase's dependency bubbles; it reuses the scores PSUM ring.
  - x chunks stream in 4-contraction-tile DMAs (few, large transfers --
    the cost model serializes descriptor generation per DMA instruction).
"""

import os
from contextlib import ExitStack

import numpy as np

import concourse.bass as bass
import concourse.tile as tile
from concourse import bass_isa
from concourse import bacc, mybir
from concourse.bass_utils import run_bass_kernel_spmd

# problem constants
B, T, C = 2, 2048, 2048
H, HKV, D = 16, 4, 128
GROUP = H // HKV           # 4 q heads per kv head
THETA = 1000000.0
SCALE = D ** -0.5

P = 128                    # partitions
TCH = 512                  # t-chunk (matmul moving free dim)
NJT = T // TCH             # 4 t-chunks
NK = C // P                # 16 contraction tiles
NH = GROUP                 # 4 local q heads
N_CORES = 8

F32 = mybir.dt.float32
AF = mybir.ActivationFunctionType
ALU = mybir.AluOpType


def _sb_dt(mode):
    return {"f16": mybir.dt.float16, "bf16": mybir.dt.bfloat16}[mode]


def _np_dt(mode):
    if mode == "bf16":
        import ml_dtypes
        return ml_dtypes.bfloat16
    return np.float16


def build_program(mode="f16", phases="ABC", variant=""):
    """Build and compile the per-core Bass program. Returns nc."""
    sb_dt = _sb_dt(mode)

    nc = bacc.Bacc("TRN2", target_bir_lowering=False, debug=False)

    xT_d = nc.dram_tensor("xT", [C, T], sb_dt, kind="ExternalInput").ap()
    wq_d = nc.dram_tensor("wqT", [C, NH * D], sb_dt, kind="ExternalInput").ap()
    wk_d = nc.dram_tensor("wkT", [C, D], sb_dt, kind="ExternalInput").ap()
    wv_d = nc.dram_tensor("wvT", [C, D], sb_dt, kind="ExternalInput").ap()
    wo_d = nc.dram_tensor("woT", [NH * D, C], sb_dt, kind="ExternalInput").ap()
    cos_d = nc.dram_tensor("cosT", [P, T], sb_dt, kind="ExternalInput").ap()
    sin_d = nc.dram_tensor("sinT", [P, T], sb_dt, kind="ExternalInput").ap()
    tri_d = nc.dram_tensor("triT", [P, P], sb_dt, kind="ExternalInput").ap()
    y_d = nc.dram_tensor("y", [T, C], sb_dt, kind="ExternalOutput").ap()

    with tile.TileContext(nc) as tc, ExitStack() as ctx:
        wpool = ctx.enter_context(tc.tile_pool(name="weights", bufs=1))
        tpool = ctx.enter_context(tc.tile_pool(name="tables", bufs=1))
        state = ctx.enter_context(tc.tile_pool(name="state", bufs=1))
        xpool = ctx.enter_context(tc.tile_pool(name="xsub", bufs=2))
        qkp = ctx.enter_context(tc.tile_pool(name="qkstage", bufs=3))
        ropep = ctx.enter_context(tc.tile_pool(name="rope", bufs=2))
        esp = ctx.enter_context(tc.tile_pool(name="es", bufs=8))
        dnp = ctx.enter_context(tc.tile_pool(name="dn", bufs=2))
        ysp = ctx.enter_context(tc.tile_pool(name="ys", bufs=4))
        psA = ctx.enter_context(tc.tile_pool(name="psA", bufs=2, space="PSUM"))
        psS = ctx.enter_context(tc.tile_pool(name="psS", bufs=4, space="PSUM"))
        psO = ctx.enter_context(tc.tile_pool(name="psO", bufs=2, space="PSUM"))

        # ---- weight / table loads -------------------------------------
        wq_sb = wpool.tile([P, NK, NH * D], sb_dt, tag="wq")
        wk_sb = wpool.tile([P, NK, D], sb_dt, tag="wk")
        wv_sb = wpool.tile([P, NK, D], sb_dt, tag="wv")
        wq_chunks = [(0, 1), (1, 2)] + [(k, k + 2) for k in range(2, NK, 2)]
        for k0, k1 in wq_chunks:
            nc.scalar.dma_start(
                wq_sb[:, k0:k1, :],
                wq_d[k0 * P:k1 * P, :].rearrange("(ko p) o -> p ko o", p=P))
        nc.gpsimd.dma_start(wk_sb[:], wk_d.rearrange("(ko p) o -> p ko o", p=P))
        nc.gpsimd.dma_start(wv_sb[:], wv_d.rearrange("(ko p) o -> p ko o", p=P))

        cos_sb = tpool.tile([P, T], sb_dt, tag="cos")
        sin_sb = tpool.tile([P, T], sb_dt, tag="sin")
        nc.gpsimd.dma_start(cos_sb[:], cos_d[:])
        nc.gpsimd.dma_start(sin_sb[:], sin_d[:])
        tri_sb = tpool.tile([P, P], sb_dt, tag="tri")
        nc.gpsimd.dma_start(tri_sb[:], tri_d[:])
        # output-projection weights; loaded per-jc slice during B(0) so the
        # transfers don't compete with the jt0 x-chunk stream
        wo_sb = wpool.tile([P, NH, C], sb_dt, tag="wo")

        qrot = state.tile([P, NH, T], sb_dt, tag="qrot")
        krot = state.tile([P, T], sb_dt, tag="krot")
        v_sb = state.tile([P, T // P, D], sb_dt, tag="v")
        ot_sb = state.tile([P, NH, T], sb_dt, tag="ot")

        def load_x(jt):
            """Stream one [C, TCH] x chunk in 4 four-k-tile DMAs."""
            xch = xpool.tile([P, NK, TCH], sb_dt, tag="x", name=f"x{jt}")
            for q in range(4):
                nc.sync.dma_start(
                    xch[:, 4 * q:4 * (q + 1), :],
                    xT_d[4 * q * P:4 * (q + 1) * P,
                         jt * TCH:(jt + 1) * TCH].rearrange(
                        "(ko p) t -> p ko t", p=P))
            return xch

        def rope(acc_ps, qsb, out_ap, jt):
            # the half-swap runs as an SBUF->SBUF DMA (engines cannot read
            # two SBUF operands at different base partitions), keeping all
            # DVE ops on the fast 2-byte path
            ch = slice(jt * TCH, (jt + 1) * TCH)
            qsw = qkp.tile([P, TCH], sb_dt, tag="qsw")
            nc.sync.dma_start(qsw[0:64, :], qsb[64:128, :])
            nc.sync.dma_start(qsw[64:128, :], qsb[0:64, :])
            m1 = ropep.tile([P, TCH], sb_dt, tag="m1")
            m2 = ropep.tile([P, TCH], sb_dt, tag="m2")
            nc.vector.tensor_tensor(m1[:], qsb[:], cos_sb[:, ch], ALU.mult)
            nc.vector.tensor_tensor(m2[:], qsw[:], sin_sb[:, ch], ALU.mult)
            nc.vector.tensor_tensor(out_ap, m1[:], m2[:], ALU.add)

        def w_slice(o, k):
            # output index o: 0..3 = q heads, 4 = k
            if o < NH:
                return wq_sb[:, k, o * D:(o + 1) * D]
            return wk_sb[:, k, :]

        def emit_C(jt_c, tl, last=False):
            """Output projection for row-block tt = 4*jt_c + tl."""
            tt = jt_c * (TCH // P) + tl
            for jc in range(NJT):
                yp = psS.tile([P, TCH], F32, tag="s", name="yp")
                for h in range(NH):
                    nc.tensor.matmul(
                        yp[:],
                        ot_sb[:, h, tt * P:(tt + 1) * P],
                        wo_sb[:, h, jc * TCH:(jc + 1) * TCH],
                        start=(h == 0), stop=(h == NH - 1))
                ys = ysp.tile([P, TCH], sb_dt, tag="ys")
                if jc % 2 == 0:
                    nc.vector.tensor_copy(ys[:], yp[:])
                else:
                    nc.scalar.activation(ys[:], yp[:], AF.Copy)
                nc.sync.dma_start(
                    y_d[tt * P:(tt + 1) * P, jc * TCH:(jc + 1) * TCH], ys[:])

        xch_cur = load_x(0)
        for jt in range(NJT):
            ch = slice(jt * TCH, (jt + 1) * TCH)
            xs = xch_cur

            # ---- A(jt): q/k projections + RoPE, v in [t, d] layout ----
            if "A" in phases:
                if jt == 0:
                    # k-outer: consume wq chunks as they stream in
                    accs = [(psA if o < 2 else psS).tile(
                        [P, TCH], F32, tag=("acc" if o < 2 else "s"),
                        name=f"acc{o}") for o in range(5)]
                    for k in range(NK):
                        for o in range(5):
                            nc.tensor.matmul(
                                accs[o][:], w_slice(o, k), xs[:, k, :],
                                start=(k == 0), stop=(k == NK - 1))
                    for o in (4, 0, 1, 2, 3):
                        qsb = qkp.tile([P, TCH], sb_dt, tag="qk")
                        nc.scalar.activation(qsb[:], accs[o][:], AF.Copy)
                        rope(accs[o], qsb, krot[:, ch] if o == 4
                             else qrot[:, o, ch], jt)
                else:
                    # output-major over the rotating psA ring
                    for o in (4, 0, 1, 2, 3):
                        acc = psA.tile([P, TCH], F32, tag="acc")
                        for k in range(NK):
                            nc.tensor.matmul(
                                acc[:], w_slice(o, k), xs[:, k, :],
                                start=(k == 0), stop=(k == NK - 1))
                        qsb = qkp.tile([P, TCH], sb_dt, tag="qk")
                        nc.scalar.activation(qsb[:], acc[:], AF.Copy)
                        rope(acc, qsb, krot[:, ch] if o == 4
                             else qrot[:, o, ch], jt)

                # v: x chunk stationary, wv moving
                for tl in range(TCH // P):
                    tt = jt * (TCH // P) + tl
                    vt_ps = psA.tile([P, D], F32, tag="acc", name="vt")
                    for k in range(NK):
                        nc.tensor.matmul(
                            vt_ps[:], xs[:, k, tl * P:(tl + 1) * P],
                            wv_sb[:, k, :],
                            start=(k == 0), stop=(k == NK - 1))
                    nc.scalar.activation(v_sb[:, tt, :], vt_ps[:], AF.Copy)

            # ---- B(jt): attention; C(jt-1) interleaved per head -------
            if "B" in phases:
                if jt == 0:
                    for jc in range(NJT):
                        nc.scalar.dma_start(
                            wo_sb[:, :, jc * TCH:(jc + 1) * TCH],
                            wo_d[:, jc * TCH:(jc + 1) * TCH].rearrange(
                                "(h p) c -> p h c", p=P))
                if jt + 1 < NJT:
                    xch_cur = load_x(jt + 1)
                njs = 4 * jt + 4
                for h in range(NH):
                    ot_ps = psO.tile([P, TCH], F32, tag="ot")
                    qch = qrot[:, h, ch]
                    dn = dnp.tile([P, TCH], sb_dt, tag="dn")

                    def emit_pv(es, js, toff, njs=njs, ot_ps=ot_ps):
                        nc.tensor.matmul(
                            ot_ps[:, toff:], v_sb[:, js, :], es[:, toff:],
                            start=(js == 0), stop=(js == njs - 1),
                            skip_group_check=True)

                    pend = []  # deferred so exp latency is hidden
                    for js in range(njs):
                        toff = P * (js - 4 * jt) if js >= 4 * jt else 0
                        s_ps = psS.tile([P, TCH], F32, tag="s")
                        nc.tensor.matmul(
                            s_ps[:, toff:], krot[:, js * P:(js + 1) * P],
                            qch[:, toff:], start=True, stop=True,
                            skip_group_check=True)
                        es = esp.tile([P, TCH], sb_dt, tag="es")
                        nc.scalar.activation(
                            es[:, toff:], s_ps[:, toff:], AF.Exp, scale=SCALE)
                        if js >= 4 * jt:  # diagonal: triangular mask block
                            nc.vector.tensor_tensor(
                                es[:, toff:toff + P], es[:, toff:toff + P],
                                tri_sb[:], ALU.mult)
                        if js == 0:
                            nc.vector.tensor_copy(dn[:], es[:])
                        else:
                            nc.vector.tensor_tensor(
                                dn[:, toff:], dn[:, toff:], es[:, toff:],
                                ALU.add)
                        if len(pend) >= 4:
                            emit_pv(*pend.pop(0))
                        pend.append((es, js, toff))
                    for p_ in pend:
                        emit_pv(*p_)

                    den = dnp.tile([P, TCH], sb_dt, tag="den")
                    nc.gpsimd.partition_all_reduce(
                        den[:], dn[:], 128, bass_isa.ReduceOp.add)
                    rec = dnp.tile([P, TCH], sb_dt, tag="rec")
                    with nc.allow_low_precision(reason="fp16 softmax denom"):
                        nc.vector.reciprocal(rec[:], den[:])
                    nc.vector.tensor_tensor(
                        ot_sb[:, h, ch], ot_ps[:], rec[:], ALU.mult)

                    # fill attention bubbles with prev chunk's out-proj
                    if "C" in phases and jt > 0:
                        emit_C(jt - 1, h)

        # ---- C(3): final chunk's output projection --------------------
        if "C" in phases:
            for tl in range(TCH // P):
                emit_C(NJT - 1, tl, last=True)

    nc.compile()
    return nc


def host_prep(x, wq, wk, wv, wo, mode="f16"):
    """Build the 8 per-core input maps (numpy, host-side reshuffles only)."""
    ndt = _np_dt(mode)
    x = np.asarray(x, dtype=np.float32)
    wq = np.asarray(wq, dtype=np.float32)
    wk = np.asarray(wk, dtype=np.float32)
    wv = np.asarray(wv, dtype=np.float32)
    wo = np.asarray(wo, dtype=np.float32)

    # RoPE even/odd grouping permutation within each head
    perm = np.concatenate([np.arange(0, D, 2), np.arange(1, D, 2)])

    # rope tables, transposed layout [d, t]
    inv_freq = (1.0 / THETA ** (np.arange(0, D, 2, dtype=np.float32) / D)
                ).astype(np.float32)
    pos = np.arange(T, dtype=np.float32)
    freqs = pos[:, None] * inv_freq[None, :]          # [T, 64] f32
    cos_t = np.cos(freqs).astype(np.float32).T        # [64, T]
    sin_t = np.sin(freqs).astype(np.float32).T        # [64, T]
    cosT = np.concatenate([cos_t, cos_t], axis=0)     # [128, T]
    sinT = np.concatenate([-sin_t, sin_t], axis=0)    # [128, T]

    # triangular multiplicative mask for diagonal blocks: allow f >= p
    f = np.arange(P)[None, :]
    p = np.arange(P)[:, None]
    tri = (f >= p).astype(np.float32)

    xTs = [np.ascontiguousarray(x[b].T).astype(ndt) for b in range(B)]

    in_maps = []
    for c in range(N_CORES):
        b, g = divmod(c, GROUP)
        rows = []
        for hh in range(NH):
            h = g * GROUP + hh
            rows.append(wq[h * D + perm, :])
        wq_g = np.concatenate(rows, axis=0)                  # [512, C]
        wk_g = wk[g * D + perm, :]                           # [128, C]
        wv_g = wv[g * D:(g + 1) * D, :]                      # [128, C]
        wo_g = wo[:, g * NH * D:(g + 1) * NH * D]            # [C, 512]

        in_maps.append({
            "xT": xTs[b],
            "wqT": np.ascontiguousarray(wq_g.T).astype(ndt),
            "wkT": np.ascontiguousarray(wk_g.T).astype(ndt),
            "wvT": np.ascontiguousarray(wv_g.T).astype(ndt),
            "woT": np.ascontiguousarray(wo_g.T).astype(ndt),
            "cosT": cosT.astype(ndt),
            "sinT": sinT.astype(ndt),
            "triT": tri.astype(ndt),
        })
    return in_maps


_CACHE = {}


def _get_program(mode):
    if mode not in _CACHE:
        _CACHE[mode] = build_program(mode)
    return _CACHE[mode]


def kernel(x, mask, wq, wk, wv, wo):
    mode = os.environ.get("BASS_ATTN_MODE", "f16")
    nc = _get_program(mode)
    in_maps = host_prep(x, wq, wk, wv, wo, mode)
    res = run_bass_kernel_spmd(nc, in_maps, list(range(N_CORES))).results
    out = np.zeros((B, T, C), dtype=np.float32)
    for c in range(N_CORES):
        out[c // GROUP] += res[c]["y"]
    return out
